# revision 1
# baseline (speedup 1.0000x reference)
"""NeuronPool (moe_routing) Trainium2 kernel.

Expert-parallel over 8 NeuronCores: core c computes neurons [8c, 8c+8) for the
full batch, host concatenates along the neuron axis.

Per-core pipeline (all shapes per core):
  x = [proj | hist_broadcast]  (built on device, stored transposed as 18
      [128,32] f32r tiles so the batch stays on the PSUM partition dim)
  A(n), per neuron:
      psum1[32,512] = sel(n).T @ b1_rows  +  sum_k xT[k].T @ W1[n,k]   (f32r;
          biases/gamma/beta live one-neuron-per-partition and broadcast via a
          K=8 one-hot selector matmul)
      h1 = gelu(psum1)                 -> PE-transpose -> h1T [128,32] x4
      psum2[32,512] = bias + sum_j h1T[j].T @ W2[n,j]
      h2 = gelu(psum2)                 -> PE-transpose -> h2T
      psum3[32,256] = bias + sum_j h2T[j].T @ W3[n,j]
      y = copy(psum3) + row sums (ACT accum_out); yc = y - mean; ssq(yc)
  B(n), emitted one neuron behind A so it pipelines instead of trailing:
      inv_std = 1/sqrt(ssq/D + eps); out = yc*inv_std*(gamma*mod) + beta*mod
The last two neurons' weight DMAs interleave with the layer pipeline so the
final arriving bytes (W3 of the last neuron) feed the shortest compute chain.
Weights stream HBM->SBUF as ~1MiB SWDGE DMAs with an inline fp32->float32r
cast (float32r matmuls run at 4x the fp32 rate; ~1.5e-4 relative rounding).
Measured: 156.3 us HW exec per core, relative error 2.7e-4 vs fp32 reference.
"""
import math
import numpy as np
from contextlib import ExitStack

import concourse.bass as bass
import concourse.tile as tile
from concourse import bacc, mybir
from concourse.bass_utils import run_bass_kernel_spmd

N_CORES = 8
B = 32          # batch
D = 256         # model dim
HIST = 8
HID = 512
N_NEURONS = 64
NPC = N_NEURONS // N_CORES  # 8 neurons per core
IN_DIM = D * (1 + HIST)     # 2304
KC1 = IN_DIM // 128         # 18 contraction chunks for GEMM1
KC2 = HID // 128            # 4 chunks for GEMM2/GEMM3
LN_EPS = 1e-5
FMIN, FMAX = 0.5, 40.0
TICK_INTERVAL = 0.1

f32 = mybir.dt.float32
f32r = mybir.dt.float32r

# packed per-neuron row layout (columns in bvec8: one SBUF partition per
# neuron, broadcast into PSUM via a K=8 one-hot selector matmul)
B1_OFF = 0
B2_OFF = B1_OFF + HID
B3_OFF = B2_OFF + HID
GM_OFF = B3_OFF + D
BM_OFF = GM_OFF + D
BVEC_LEN = BM_OFF + D

_CACHE = {}


def _build_program():
    nc = bacc.Bacc("TRN2", target_bir_lowering=False, debug=False,
                   num_devices=N_CORES)

    emb = nc.dram_tensor("emb", [B, D], f32, kind="ExternalInput").ap()
    wp = nc.dram_tensor("wp", [D, D], f32, kind="ExternalInput").ap()
    bpd = nc.dram_tensor("bpd", [128, 2], f32, kind="ExternalInput").ap()
    histd = nc.dram_tensor("histd", [16, 128], f32, kind="ExternalInput").ap()
    eyed = nc.dram_tensor("eyed", [32, 32], f32, kind="ExternalInput").ap()
    w1d = nc.dram_tensor("w1d", [NPC, 128, KC1, HID], f32, kind="ExternalInput").ap()
    w2d = nc.dram_tensor("w2d", [NPC, 128, KC2, HID], f32, kind="ExternalInput").ap()
    w3d = nc.dram_tensor("w3d", [NPC, 128, KC2, D], f32, kind="ExternalInput").ap()
    bvecd = nc.dram_tensor("bvecd", [NPC, BVEC_LEN], f32, kind="ExternalInput").ap()
    sel8d = nc.dram_tensor("sel8d", [NPC, NPC * B], f32, kind="ExternalInput").ap()
    out = nc.dram_tensor("out", [B, NPC, D], f32, kind="ExternalOutput").ap()

    GELU = mybir.ActivationFunctionType.Gelu
    COPY = mybir.ActivationFunctionType.Copy
    SQUARE = mybir.ActivationFunctionType.Square
    SQRT = mybir.ActivationFunctionType.Sqrt

    with tile.TileContext(nc) as tc, ExitStack() as ctx:
        # SBUF pools
        cst = ctx.enter_context(tc.tile_pool(name="cst", bufs=1))
        xtp = ctx.enter_context(tc.tile_pool(name="xtp", bufs=KC1))
        w1p = ctx.enter_context(tc.tile_pool(name="w1p", bufs=8))
        w23p = ctx.enter_context(tc.tile_pool(name="w23p", bufs=6))
        htp = ctx.enter_context(tc.tile_pool(name="htp", bufs=16))
        hp = ctx.enter_context(tc.tile_pool(name="hp", bufs=4))
        ysp = ctx.enter_context(tc.tile_pool(name="ysp", bufs=NPC))
        rsp = ctx.enter_context(tc.tile_pool(name="rsp", bufs=NPC))
        yp = ctx.enter_context(tc.tile_pool(name="yp", bufs=10))
        stp = ctx.enter_context(tc.tile_pool(name="stp", bufs=12))
        # PSUM pools (8 banks total: 3 + 3 + 2)
        accp = ctx.enter_context(tc.tile_pool(name="accp", bufs=3, space="PSUM"))
        trp = ctx.enter_context(tc.tile_pool(name="trp", bufs=3, space="PSUM"))
        gbp = ctx.enter_context(tc.tile_pool(name="gbp", bufs=2, space="PSUM"))

        # ---- constants ----
        eye = cst.tile([32, 32], f32, tag="eye")
        nc.sync.dma_start(out=eye[:], in_=eyed)
        onesf = cst.tile([1, 32], f32, tag="onesf")
        nc.vector.memset(onesf[:], 1.0)
        onesr = cst.tile([1, 32], f32r, tag="onesr")
        nc.vector.tensor_copy(onesr[:], onesf[:])
        onesb = cst.tile([128, 32], f32, tag="onesb")
        nc.vector.memset(onesb[:], 1.0)
        epst = cst.tile([B, 1], f32, tag="epst")
        nc.vector.memset(epst[:], LN_EPS)
        bpt = cst.tile([128, 2], f32, tag="bpt")
        nc.sync.dma_start(out=bpt[:], in_=bpd)
        bvec = cst.tile([NPC, BVEC_LEN], f32r, tag="bvec")
        nc.gpsimd.dma_start(out=bvec[:], in_=bvecd)
        sel8 = cst.tile([NPC, NPC * B], f32r, tag="sel8")
        nc.gpsimd.dma_start(out=sel8[:], in_=sel8d)

        # K=8 one-hot selector: sel8[:, 32n:32n+32].T @ bvec[:, off:off+w]
        # broadcasts neuron n's packed row across the 32 batch partitions
        def selcol(n):
            return sel8[:, n * B:(n + 1) * B]

        def b1row(n):
            return bvec[:, B1_OFF:B1_OFF + HID]

        def b2row(n):
            return bvec[:, B2_OFF:B2_OFF + HID]

        def b3row(n):
            return bvec[:, B3_OFF:B3_OFF + D]

        def gmrow(n):
            return bvec[:, GM_OFF:GM_OFF + D]

        def bmrow(n):
            return bvec[:, BM_OFF:BM_OFF + D]

        # ---- x setup: xT chunks [128, 32] f32r, k = 0..17 ----
        xT = []

        # proj part: projT = Wp.T @ emb.T + bp, chunks 0..1
        xe = cst.tile([B, D], f32, tag="xe")
        nc.sync.dma_start(out=xe[:], in_=emb)
        wpt = cst.tile([128, 2, D], f32r, tag="wpt")
        nc.gpsimd.dma_start(out=wpt[:], in_=wp.rearrange("(c p) d -> p c d", p=128))
        xeT = []
        for k in range(2):
            pt = trp.tile([128, 32], f32, tag="tr")
            nc.tensor.transpose(pt[:], xe[:, k * 128:(k + 1) * 128], eye[:])
            st = cst.tile([128, 32], f32r, tag=f"xeT{k}")
            nc.vector.tensor_copy(st[:], pt[:])
            xeT.append(st)
        for m in range(2):
            pp = trp.tile([128, 32], f32, tag="tr")
            for k in range(2):
                nc.tensor.matmul(pp[:], wpt[:, k, m * 128:(m + 1) * 128], xeT[k][:],
                                 start=(k == 0), stop=(k == 1))
            xt = xtp.tile([128, 32], f32r, tag="xt")
            nc.vector.tensor_scalar_add(xt[:], pp[:], bpt[:, m:m + 1])
            xT.append(xt)

        # hist part: chunks 2..17 broadcast across batch
        ht = cst.tile([16, 128], f32, tag="ht")
        nc.sync.dma_start(out=ht[:], in_=histd)
        pt = trp.tile([128, 16], f32, tag="tr")
        nc.tensor.transpose(pt[:], ht[:], eye[0:16, 0:16])
        histT = cst.tile([128, 16], f32, tag="histT")
        nc.vector.tensor_copy(histT[:], pt[:])
        for c in range(16):
            xt = xtp.tile([128, 32], f32r, tag="xt")
            nc.vector.tensor_scalar_mul(xt[:], onesb[:], histT[:, c:c + 1])
            xT.append(xt)

        # ---- main pipeline: emit_A(n) = GEMMs + gelus + centered y stats;
        # emit_B(n) = inv_std + modulated affine + output DMA.  B(n-1) is
        # emitted after A(n) so every engine keeps pipelined work and the
        # kernel tail is only B(last).
        ycs = {}
        stats = {}

        def dma_w1(n):
            w1t = []
            for s in range(4):
                t = w1p.tile([128, 4, HID], f32r, tag="w1")
                nc.gpsimd.dma_start(out=t[:], in_=w1d[n][:, 4 * s:4 * s + 4, :])
                w1t.append(t)
            t = w1p.tile([128, 4, HID], f32r, tag="w1")
            nc.gpsimd.dma_start(out=t[:, 0:2, :], in_=w1d[n][:, 16:18, :])
            w1t.append(t)
            return w1t

        def dma_w2(n):
            w2t = w23p.tile([128, KC2, HID], f32r, tag="w23")
            nc.gpsimd.dma_start(out=w2t[:], in_=w2d[n])
            return w2t

        def dma_w3(n):
            w3t = w23p.tile([128, KC2, D], f32r, tag="w23")
            nc.gpsimd.dma_start(out=w3t[:], in_=w3d[n])
            return w3t

        def transpose4(h):
            hT = []
            for j in range(KC2):
                pt = trp.tile([128, 32], f32, tag="tr")
                nc.tensor.transpose(pt[:], h[:, j * 128:(j + 1) * 128], eye[:])
                st = htp.tile([128, 32], f32r, tag="hT")
                nc.vector.tensor_copy(st[:], pt[:])
                hT.append(st)
            return hT

        def gemm1(n, w1t):
            p1 = accp.tile([B, HID], f32, tag="acc")
            nc.tensor.matmul(p1[:], selcol(n), b1row(n), start=True, stop=False)
            for k in range(KC1):
                nc.tensor.matmul(p1[:], xT[k][:], w1t[k // 4][:, k % 4, :],
                                 start=False, stop=(k == KC1 - 1))
            h1 = hp.tile([B, HID], f32, tag="h")
            nc.scalar.activation(h1[:], p1[:], GELU)
            return transpose4(h1)

        def gemm2(n, w2t, h1T):
            p2 = accp.tile([B, HID], f32, tag="acc")
            nc.tensor.matmul(p2[:], selcol(n), b2row(n), start=True, stop=False)
            for j in range(KC2):
                nc.tensor.matmul(p2[:], h1T[j][:], w2t[:, j, :],
                                 start=False, stop=(j == KC2 - 1))
            h2 = hp.tile([B, HID], f32, tag="h")
            nc.scalar.activation(h2[:], p2[:], GELU)
            return transpose4(h2)

        def gemm3(n, w3t, h2T):
            p3 = accp.tile([B, D], f32, tag="acc")
            nc.tensor.matmul(p3[:], selcol(n), b3row(n), start=True, stop=False)
            for j in range(KC2):
                nc.tensor.matmul(p3[:], h2T[j][:], w3t[:, j, :],
                                 start=False, stop=(j == KC2 - 1))

            # center y and accumulate sum(yc^2), all on DVE (no ACT table):
            #   rs = sum(y); yc = y - rs/D; ssq = sum(yc*yc)
            y = yp.tile([B, D], f32, tag="y")
            rs = rsp.tile([B, 1], f32, tag="rs")
            nc.scalar.activation(y[:], p3[:], COPY, accum_out=rs[:])
            nmu = stp.tile([B, 1], f32, tag="st")
            nc.vector.tensor_scalar_mul(nmu[:], rs[:], -1.0 / D)
            yc = ysp.tile([B, D], f32, tag="ys")
            nc.vector.tensor_scalar_add(yc[:], y[:], nmu[:])
            sqs = yp.tile([B, D], f32, tag="y")
            ssq = stp.tile([B, 1], f32, tag="st")
            nc.scalar.activation(sqs[:], yc[:], SQUARE, accum_out=ssq[:])
            ycs[n] = yc
            stats[n] = ssq

        def emit_A(n):
            # weights stream in consumption order: W1, W2, W3
            w1t = dma_w1(n)
            w2t = dma_w2(n)
            w3t = dma_w3(n)
            h1T = gemm1(n, w1t)
            h2T = gemm2(n, w2t, h1T)
            gemm3(n, w3t, h2T)

        def emit_B(n):
            yc, ssq = ycs[n], stats[n]
            std = stp.tile([B, 1], f32, tag="st")
            nc.scalar.activation(std[:], ssq[:], SQRT, bias=epst[:], scale=1.0 / D)
            inv = stp.tile([B, 1], f32, tag="st")
            nc.vector.reciprocal(inv[:], std[:])

            gb = gbp.tile([B, 2 * D], f32, tag="gb")
            nc.tensor.matmul(gb[:, 0:D], selcol(n), gmrow(n), start=True, stop=True)
            nc.tensor.matmul(gb[:, D:2 * D], selcol(n), bmrow(n), start=True, stop=True)

            yg = yp.tile([B, D], f32, tag="y")
            nc.vector.scalar_tensor_tensor(
                yg[:], yc[:], inv[:], gb[:, 0:D],
                mybir.AluOpType.mult, mybir.AluOpType.mult)
            yo = yp.tile([B, D], f32, tag="y")
            nc.vector.tensor_add(yo[:], yg[:], gb[:, D:2 * D])

            nc.sync.dma_start(out=out[:, n, :], in_=yo[:])

        # Neurons 0..NPC-3 run the standard pipeline with B lagging one
        # neuron.  The last two neurons interleave their DMA stream with the
        # layer pipeline so the final arriving bytes (W3 of the last neuron)
        # feed the shortest possible compute chain (GEMM3 + LN + output):
        # pool order ... W1[p] W2[p] W1[L] W3[p] W2[L] W3[L].
        p, L = NPC - 2, NPC - 1
        for n in range(p):
            emit_A(n)
            if n > 0:
                emit_B(n - 1)

        w1p_t = dma_w1(p)
        w2p_t = dma_w2(p)
        h1Tp = gemm1(p, w1p_t)
        h2Tp = gemm2(p, w2p_t, h1Tp)
        emit_B(p - 1)
        w1L_t = dma_w1(L)
        h1TL = gemm1(L, w1L_t)
        w2L_t = dma_w2(L)
        h2TL = gemm2(L, w2L_t, h1TL)
        w3p_t = dma_w3(p)
        gemm3(p, w3p_t, h2Tp)
        w3L_t = dma_w3(L)
        gemm3(L, w3L_t, h2TL)
        emit_B(p)
        emit_B(L)

    nc.compile()
    return nc


def _get_program():
    if "nc" not in _CACHE:
        _CACHE["nc"] = _build_program()
    return _CACHE["nc"]


def _prep_in_maps(input_embedding, pre_activations, Wp, bp, W1, b1, W2, b2, W3,
                  b3, gamma, beta, tick):
    emb = np.asarray(input_embedding, dtype=np.float32)
    hist = np.asarray(pre_activations, dtype=np.float32)
    Wp = np.asarray(Wp, dtype=np.float32)
    bp = np.asarray(bp, dtype=np.float32)
    W1 = np.asarray(W1, dtype=np.float32)
    b1 = np.asarray(b1, dtype=np.float32)
    W2 = np.asarray(W2, dtype=np.float32)
    b2 = np.asarray(b2, dtype=np.float32)
    W3 = np.asarray(W3, dtype=np.float32)
    b3 = np.asarray(b3, dtype=np.float32)
    gamma = np.asarray(gamma, dtype=np.float32)
    beta = np.asarray(beta, dtype=np.float32)

    # oscillator modulation folded into gamma/beta
    i = np.arange(N_NEURONS, dtype=np.float64)
    freq = FMIN * (FMAX / FMIN) ** (i / (N_NEURONS - 1))
    phase = np.mod(i * 2.3571, 2.0 * math.pi)
    t = float(np.asarray(tick)) * TICK_INTERVAL
    mod = (1.0 + 0.5 * np.sin(2.0 * math.pi * freq * t + phase)).astype(np.float32)
    gm = (gamma * mod[:, None]).astype(np.float32)
    bm = (beta * mod[:, None]).astype(np.float32)

    histd = np.ascontiguousarray(hist.reshape(16, 128))
    bpd = np.ascontiguousarray(bp.reshape(2, 128).T)
    eyed = np.eye(32, dtype=np.float32)

    # weight layout: (n, p, k_chunk, hid) so each supertile DMA reads one
    # contiguous run per partition
    W1r = np.ascontiguousarray(
        W1.reshape(N_NEURONS, KC1, 128, HID).transpose(0, 2, 1, 3))
    W2r = np.ascontiguousarray(
        W2.reshape(N_NEURONS, KC2, 128, HID).transpose(0, 2, 1, 3))
    W3r = np.ascontiguousarray(
        W3.reshape(N_NEURONS, KC2, 128, D).transpose(0, 2, 1, 3))

    # one-hot selector: sel8[k, n*B + j] = (k == n), broadcasts bvec row n
    # across the batch partitions via a K=8 matmul
    sel8 = np.zeros((NPC, NPC * B), dtype=np.float32)
    for n in range(NPC):
        sel8[n, n * B:(n + 1) * B] = 1.0

    in_maps = []
    for c in range(N_CORES):
        s = slice(c * NPC, (c + 1) * NPC)
        bvec = np.concatenate([b1[s], b2[s], b3[s], gm[s], bm[s]], axis=1)
        in_maps.append({
            "emb": emb,
            "wp": Wp,
            "bpd": bpd,
            "histd": histd,
            "eyed": eyed,
            "w1d": W1r[s],
            "w2d": W2r[s],
            "w3d": W3r[s],
            "bvecd": np.ascontiguousarray(bvec),
            "sel8d": sel8,
        })
    return in_maps


def run(inputs, trace=False):
    nc = _get_program()
    in_maps = _prep_in_maps(**inputs)
    br = run_bass_kernel_spmd(nc, in_maps, core_ids=list(range(N_CORES)),
                              trace=trace)
    out = np.concatenate([r["out"] for r in br.results], axis=1)
    return np.ascontiguousarray(out, dtype=np.float32), br


def kernel(**inputs) -> np.ndarray:
    out, _ = run(inputs, trace=False)
    return out



# revision 6
# speedup vs baseline: 1.4361x; 1.4361x over previous
"""NeuronPool (moe_routing) Trainium2 kernel.

Expert-parallel over 8 NeuronCores: core c computes neurons [8c, 8c+8) for the
full batch, host concatenates along the neuron axis.

The kernel is HBM-bound: ~25 MB of weights stream per core.  Weights are cast
to fp16 on host (matmuls run at the full 1 cycle/row PE rate; ~5e-4 relative
rounding, well inside the fp32 envelope), halving DMA traffic vs fp32/f32r.

Per-core pipeline (all shapes per core, batch B=32 on PSUM partitions):
  x = [proj | hist]: proj = Wp.T @ embT on the PE (fp16), hist chunks arrive
      pre-broadcast from host as [128, 16, 32] (layout prep only).
  per neuron n:
      p1[32,512] = sum_k xT[k].T @ W1[n,k]      (18 fp16 matmuls)
      h1 = gelu(p1) -> PE-transpose -> h1T [128,32] x4  (fp16)
      p2[32,512] = sum_j h1T[j].T @ W2[n,j]; h2 = gelu(p2) -> h2T
      p3[32,257] = sum_j h2T[j].T @ W3aug[n,j]  (col 256 = row-sums of W3,
          so the LayerNorm mean is a free by-product of the GEMM)
      yc = p3[:,0:256] - p3[:,256]/256          (DVE)
      ssq[:,n] = sum(yc*yc)                     (one fused DVE op)
  tail: std = sqrt(ssq/D + eps) for all 8 neurons in ONE activation call
      (avoids per-neuron Gelu<->Sqrt table thrash), inv = 1/std on DVE,
      out[n] = yc * inv * mod[n] (oscillator mod folded in as an immediate).
Zero bias vectors / unit gamma / zero beta are detected on host at call time
and their device ops are skipped (generic selector-matmul paths are emitted
when the values are non-trivial, as in the original kernel).
"""
import math
import numpy as np
from contextlib import ExitStack

import concourse.bass as bass
import concourse.tile as tile
from concourse import bacc, mybir
from concourse.bass_utils import run_bass_kernel_spmd

N_CORES = 8
B = 32          # batch
D = 256         # model dim
HIST = 8
HID = 512
N_NEURONS = 64
NPC = N_NEURONS // N_CORES  # 8 neurons per core
IN_DIM = D * (1 + HIST)     # 2304
KC1 = IN_DIM // 128         # 18 contraction chunks for GEMM1
KC2 = HID // 128            # 4 chunks for GEMM2/GEMM3
W3F = D + 1                 # GEMM3 free dim: 256 outputs + row-sum column
LN_EPS = 1e-5
FMIN, FMAX = 0.5, 40.0
TICK_INTERVAL = 0.1

f32 = mybir.dt.float32
f16 = mybir.dt.float16

# packed per-neuron row layout for the generic (non-zero bias) path
B1_OFF = 0
B2_OFF = B1_OFF + HID
B3_OFF = B2_OFF + HID
GM_OFF = B3_OFF + W3F
BM_OFF = GM_OFF + D
BVEC_LEN = BM_OFF + D

_CACHE = {}


def _build_program(flags):
    # flags: (b1_zero, b2_zero, b3_zero, gamma_one, beta_zero)
    b1z, b2z, b3z, g1, bz = flags
    nc = bacc.Bacc("TRN2", target_bir_lowering=False, debug=False,
                   num_devices=N_CORES)

    embTd = nc.dram_tensor("embTd", [128, 2, B], f16, kind="ExternalInput").ap()
    wpd = nc.dram_tensor("wpd", [128, 2, D], f16, kind="ExternalInput").ap()
    bpd = nc.dram_tensor("bpd", [128, 2], f32, kind="ExternalInput").ap()
    histd = nc.dram_tensor("histd", [128, HIST * 2, B], f16, kind="ExternalInput").ap()
    eyed = nc.dram_tensor("eyed", [32, 32], f16, kind="ExternalInput").ap()
    w1d = nc.dram_tensor("w1d", [NPC, 128, KC1, HID], f16, kind="ExternalInput").ap()
    w2d = nc.dram_tensor("w2d", [NPC, 128, KC2, HID], f16, kind="ExternalInput").ap()
    w3d = nc.dram_tensor("w3d", [NPC, 128, KC2, W3F], f16, kind="ExternalInput").ap()
    bvecd = nc.dram_tensor("bvecd", [NPC, BVEC_LEN], f16, kind="ExternalInput").ap()
    sel8d = nc.dram_tensor("sel8d", [NPC, NPC * B], f16, kind="ExternalInput").ap()
    modd = nc.dram_tensor("modd", [B, NPC], f32, kind="ExternalInput").ap()
    out = nc.dram_tensor("out", [B, NPC, D], f32, kind="ExternalOutput").ap()

    GELU = mybir.ActivationFunctionType.Gelu
    SQRT = mybir.ActivationFunctionType.Sqrt
    MULT = mybir.AluOpType.mult

    with tile.TileContext(nc) as tc, ExitStack() as ctx:
        cst = ctx.enter_context(tc.tile_pool(name="cst", bufs=1))
        w1p = ctx.enter_context(tc.tile_pool(name="w1p", bufs=8))
        w23p = ctx.enter_context(tc.tile_pool(name="w23p", bufs=8))
        htp = ctx.enter_context(tc.tile_pool(name="htp", bufs=16))
        hp = ctx.enter_context(tc.tile_pool(name="hp", bufs=4))
        ysp = ctx.enter_context(tc.tile_pool(name="ysp", bufs=NPC))
        yop = ctx.enter_context(tc.tile_pool(name="yop", bufs=4))
        stp = ctx.enter_context(tc.tile_pool(name="stp", bufs=4))
        scp = ctx.enter_context(tc.tile_pool(name="scp", bufs=2))
        accp = ctx.enter_context(tc.tile_pool(name="accp", bufs=4, space="PSUM"))
        trp = ctx.enter_context(tc.tile_pool(name="trp", bufs=4, space="PSUM"))

        need_sel = not (b1z and b2z and b3z and g1 and bz)

        # ---- constants / inputs ----
        eye = cst.tile([32, 32], f16, tag="eye")
        nc.sync.dma_start(out=eye[:], in_=eyed)
        epst = cst.tile([B, 1], f32, tag="epst")
        nc.vector.memset(epst[:], LN_EPS)
        embT = cst.tile([128, 2, B], f16, tag="embT")
        nc.sync.dma_start(out=embT[:], in_=embTd)
        wpt = cst.tile([128, 2, D], f16, tag="wpt")
        nc.sync.dma_start(out=wpt[:], in_=wpd)
        bpt = cst.tile([128, 2], f32, tag="bpt")
        nc.sync.dma_start(out=bpt[:], in_=bpd)
        histb = cst.tile([128, HIST * 2, B], f16, tag="histb")
        nc.sync.dma_start(out=histb[:], in_=histd)
        modt = cst.tile([B, NPC], f32, tag="modt")
        nc.sync.dma_start(out=modt[:], in_=modd)
        if need_sel:
            bvec = cst.tile([NPC, BVEC_LEN], f16, tag="bvec")
            nc.sync.dma_start(out=bvec[:], in_=bvecd)
            sel8 = cst.tile([NPC, NPC * B], f16, tag="sel8")
            nc.sync.dma_start(out=sel8[:], in_=sel8d)

            def selcol(n):
                return sel8[:, n * B:(n + 1) * B]

        # ---- weight streaming (gpsimd SWDGE ring, ~1-2 MiB chunks) ----
        def dma_w1(n):
            wa = w1p.tile([128, 9, HID], f16, tag="w1")
            nc.gpsimd.dma_start(out=wa[:], in_=w1d[n][:, 0:9, :])
            wb = w1p.tile([128, 9, HID], f16, tag="w1")
            nc.gpsimd.dma_start(out=wb[:], in_=w1d[n][:, 9:18, :])
            return (wa, wb)

        def dma_w2(n):
            w2t = w23p.tile([128, KC2, HID], f16, tag="w2")
            nc.gpsimd.dma_start(out=w2t[:], in_=w2d[n])
            return w2t

        def dma_w3(n):
            w3t = w23p.tile([128, KC2, W3F], f16, tag="w3")
            nc.gpsimd.dma_start(out=w3t[:], in_=w3d[n])
            return w3t

        # ---- x setup: 18 lhsT chunks [128, 32] f16 ----
        # proj: projT[m] = sum_k wpt[:,k,m*128:+128].T @ embT[:,k,:] (+ bp)
        xT = []
        for m in range(2):
            pp = trp.tile([128, 32], f32, tag="tr")
            for k in range(2):
                nc.tensor.matmul(pp[:], wpt[:, k, m * 128:(m + 1) * 128],
                                 embT[:, k, :], start=(k == 0), stop=(k == 1))
            xt = cst.tile([128, 32], f16, tag=f"xt{m}")
            nc.vector.tensor_scalar_add(xt[:], pp[:], bpt[:, m:m + 1])
            xT.append(xt)

        def xchunk(k):
            if k < 2:
                return xT[k][:]
            return histb[:, k - 2, :]

        # ---- per-neuron pipeline ----
        ycs = {}
        ssq8 = cst.tile([B, NPC], f32, tag="ssq8")

        def transpose4(h):
            hT = []
            for j in range(KC2):
                pt = trp.tile([128, 32], f16, tag="tr")
                nc.tensor.transpose(pt[:], h[:, j * 128:(j + 1) * 128], eye[:])
                st = htp.tile([128, 32], f16, tag="hT")
                if j % 2 == 0:
                    nc.vector.tensor_copy(st[:], pt[:])
                else:
                    nc.scalar.copy(st[:], pt[:])
                hT.append(st)
            return hT

        def gemm1(n, w1t):
            p1 = accp.tile([B, HID], f32, tag="acc")
            if not b1z:
                nc.tensor.matmul(p1[:], selcol(n), bvec[:, B1_OFF:B1_OFF + HID],
                                 start=True, stop=False)
            for k in range(KC1):
                nc.tensor.matmul(p1[:], xchunk(k), w1t[k // 9][:, k % 9, :],
                                 start=(b1z and k == 0), stop=(k == KC1 - 1))
            h1 = hp.tile([B, HID], f16, tag="h")
            nc.scalar.activation(h1[:], p1[:], GELU)
            return transpose4(h1)

        def gemm2(n, w2t, h1T):
            p2 = accp.tile([B, HID], f32, tag="acc")
            if not b2z:
                nc.tensor.matmul(p2[:], selcol(n), bvec[:, B2_OFF:B2_OFF + HID],
                                 start=True, stop=False)
            for j in range(KC2):
                nc.tensor.matmul(p2[:], h1T[j][:], w2t[:, j, :],
                                 start=(b2z and j == 0), stop=(j == KC2 - 1))
            h2 = hp.tile([B, HID], f16, tag="h")
            nc.scalar.activation(h2[:], p2[:], GELU)
            return transpose4(h2)

        def gemm3(n, w3t, h2T):
            p3 = accp.tile([B, W3F], f32, tag="acc")
            if not b3z:
                nc.tensor.matmul(p3[:], selcol(n), bvec[:, B3_OFF:B3_OFF + W3F],
                                 start=True, stop=False)
            for j in range(KC2):
                nc.tensor.matmul(p3[:], h2T[j][:], w3t[:, j, :],
                                 start=(b3z and j == 0), stop=(j == KC2 - 1))
            # col 256 of p3 = row-sum of y; center y and accumulate sum(yc^2)
            nmu = stp.tile([B, 1], f32, tag="st")
            nc.vector.tensor_scalar_mul(nmu[:], p3[:, D:D + 1], -1.0 / D)
            yc = ysp.tile([B, D], f32, tag="ys")
            nc.vector.tensor_scalar_add(yc[:], p3[:, 0:D], nmu[:])
            sq = scp.tile([B, D], f32, tag="sq")
            nc.vector.tensor_tensor(sq[:], yc[:], yc[:], MULT)
            nc.vector.tensor_reduce(ssq8[:, n:n + 1], sq[:],
                                    mybir.AxisListType.X, mybir.AluOpType.add)
            ycs[n] = yc

        def emit_A(n):
            w1t = dma_w1(n)
            w2t = dma_w2(n)
            w3t = dma_w3(n)
            h1T = gemm1(n, w1t)
            h2T = gemm2(n, w2t, h1T)
            gemm3(n, w3t, h2T)

        for n in range(NPC):
            emit_A(n)

        # ---- tail: batched LN scale + modulated affine + output ----
        std8 = stp.tile([B, NPC], f32, tag="std8")
        nc.scalar.activation(std8[:], ssq8[:], SQRT, bias=epst[:], scale=1.0 / D)
        inv8 = stp.tile([B, NPC], f32, tag="inv8")
        nc.vector.reciprocal(inv8[:], std8[:])
        if g1 and bz:
            # fold the per-neuron oscillator modulation into the inverse std
            nc.vector.tensor_tensor(inv8[:], inv8[:], modt[:], MULT)

        for n in range(NPC):
            yc = ycs[n]
            inv_n = inv8[:, n:n + 1]
            if g1 and bz:
                yo = yop.tile([B, D], f32, tag="yo")
                if n % 2 == 0:
                    nc.vector.tensor_scalar_mul(yo[:], yc[:], inv_n)
                else:
                    nc.scalar.activation(yo[:], yc[:],
                                         mybir.ActivationFunctionType.Copy,
                                         scale=inv_n)
            else:
                gb = trp.tile([B, 2 * D], f32, tag="tr")
                nc.tensor.matmul(gb[:, 0:D], selcol(n),
                                 bvec[:, GM_OFF:GM_OFF + D], start=True, stop=True)
                nc.tensor.matmul(gb[:, D:2 * D], selcol(n),
                                 bvec[:, BM_OFF:BM_OFF + D], start=True, stop=True)
                yg = yop.tile([B, D], f32, tag="yo")
                nc.vector.scalar_tensor_tensor(yg[:], yc[:], inv_n, gb[:, 0:D],
                                               MULT, MULT)
                yo = yop.tile([B, D], f32, tag="yo")
                nc.vector.tensor_add(yo[:], yg[:], gb[:, D:2 * D])
            nc.sync.dma_start(out=out[:, n, :], in_=yo[:])

    nc.compile()
    return nc


def _get_program(flags):
    if flags not in _CACHE:
        _CACHE[flags] = _build_program(flags)
    return _CACHE[flags]


def _prep(input_embedding, pre_activations, Wp, bp, W1, b1, W2, b2, W3, b3,
          gamma, beta, tick):
    emb = np.asarray(input_embedding, dtype=np.float32)
    hist = np.asarray(pre_activations, dtype=np.float32)
    Wp = np.asarray(Wp, dtype=np.float32)
    bp = np.asarray(bp, dtype=np.float32)
    W1 = np.asarray(W1, dtype=np.float32)
    b1 = np.asarray(b1, dtype=np.float32)
    W2 = np.asarray(W2, dtype=np.float32)
    b2 = np.asarray(b2, dtype=np.float32)
    W3 = np.asarray(W3, dtype=np.float32)
    b3 = np.asarray(b3, dtype=np.float32)
    gamma = np.asarray(gamma, dtype=np.float32)
    beta = np.asarray(beta, dtype=np.float32)

    # oscillator modulation: deterministic in (tick, n); folded into the
    # output affine on device (as immediates when gamma==1 and beta==0,
    # else into gamma*mod / beta*mod rows)
    i = np.arange(N_NEURONS, dtype=np.float64)
    freq = FMIN * (FMAX / FMIN) ** (i / (N_NEURONS - 1))
    phase = np.mod(i * 2.3571, 2.0 * math.pi)
    t = float(np.asarray(tick)) * TICK_INTERVAL
    mod = (1.0 + 0.5 * np.sin(2.0 * math.pi * freq * t + phase)).astype(np.float32)

    b1z = not np.any(b1)
    b2z = not np.any(b2)
    b3z = not np.any(b3)
    g1 = bool(np.all(gamma == 1.0))
    bz = not np.any(beta)

    # fp16 weight layouts: (n, partition, k_chunk, free) with contiguous
    # per-partition runs; W3 gains a row-sum column so the GEMM also
    # produces sum_d(y) for the LayerNorm mean
    W1r = np.ascontiguousarray(
        W1.reshape(N_NEURONS, KC1, 128, HID).transpose(0, 2, 1, 3)).astype(np.float16)
    W2r = np.ascontiguousarray(
        W2.reshape(N_NEURONS, KC2, 128, HID).transpose(0, 2, 1, 3)).astype(np.float16)
    W3a = np.concatenate([W3, W3.sum(axis=2, keepdims=True)], axis=2)
    W3r = np.ascontiguousarray(
        W3a.reshape(N_NEURONS, KC2, 128, W3F).transpose(0, 2, 1, 3)).astype(np.float16)

    embT = np.ascontiguousarray(emb.T.reshape(2, 128, B).transpose(1, 0, 2)).astype(np.float16)
    wpt = np.ascontiguousarray(Wp.reshape(2, 128, D).transpose(1, 0, 2)).astype(np.float16)
    bpd = np.ascontiguousarray(bp.reshape(2, 128).T)
    # hist chunks pre-broadcast across batch: [128, 16, 32]
    histc = hist.reshape(-1).reshape(16, 128)          # chunk c, dim p
    histb = np.ascontiguousarray(
        np.broadcast_to(histc.T[:, :, None], (128, 16, B))).astype(np.float16)
    eyed = np.eye(32, dtype=np.float16)

    gm = (gamma * mod[:, None]).astype(np.float32)
    bm = (beta * mod[:, None]).astype(np.float32)
    b3a = np.concatenate([b3, b3.sum(axis=1, keepdims=True)], axis=1)
    sel8 = np.zeros((NPC, NPC * B), dtype=np.float16)
    for n in range(NPC):
        sel8[n, n * B:(n + 1) * B] = 1.0

    in_maps = []
    for c in range(N_CORES):
        s = slice(c * NPC, (c + 1) * NPC)
        bvec = np.concatenate([b1[s], b2[s], b3a[s], gm[s], bm[s]],
                              axis=1).astype(np.float16)
        modrow = np.broadcast_to(mod[c * NPC:(c + 1) * NPC][None, :],
                                 (B, NPC)).astype(np.float32)
        in_maps.append({
            "modd": np.ascontiguousarray(modrow),
            "embTd": embT,
            "wpd": wpt,
            "bpd": bpd,
            "histd": histb,
            "eyed": eyed,
            "w1d": W1r[s],
            "w2d": W2r[s],
            "w3d": W3r[s],
            "bvecd": np.ascontiguousarray(bvec),
            "sel8d": sel8,
        })
    flags = (b1z, b2z, b3z, g1, bz)
    return in_maps, flags


def run(inputs, trace=False):
    in_maps, flags = _prep(**inputs)
    nc = _get_program(flags)
    br = run_bass_kernel_spmd(nc, in_maps, core_ids=list(range(N_CORES)),
                              trace=trace)
    out = np.concatenate([r["out"] for r in br.results], axis=1)
    return np.ascontiguousarray(out, dtype=np.float32), br


def kernel(**inputs) -> np.ndarray:
    out, _ = run(inputs, trace=False)
    return out


# revision 7
# speedup vs baseline: 1.5265x; 1.0630x over previous
"""NeuronPool (moe_routing) Trainium2 kernel.

Expert-parallel over 8 NeuronCores: core c computes neurons [8c, 8c+8) for the
full batch, host concatenates along the neuron axis.

The kernel is HBM-bound: ~25 MB of fp16 weights stream per core at ~420 GB/s
(weights cast on host; matmuls run at the full 1 cycle/row PE rate, ~5e-4
relative rounding).  The per-neuron MLPs are software-pipelined one neuron
deep so the PE never waits on gelu: step n runs GEMM1(n) while GEMM2/GEMM3 of
neuron n-1 fill the gelu/transpose latency.

  x = [proj | hist]: proj = Wp.T @ embT on the PE; hist chunks broadcast
      across batch on the DVE.  W1/W2 weights stream on the fast gpsimd SWDGE
      ring (16 DMA engines); small constants ride the sync ring.
  step n:  G1(n) 18mm -> [gelu h1(n)] ; G2(n-1) ; tr4 h1(n) ; [gelu h2(n-1)]
           tr4 h2(n-1) ; G3(n-1) -> LN stats(n-1) on DVE
  GEMM3 output is [32,257]: col 256 = row-sums of W3 (host-augmented), so the
  LayerNorm mean is a free by-product of the GEMM.
  LN scale: sqrt over neurons {0..5} runs while neurons 6-7 compute (their
  outputs drain early); only {6,7} remain in the tail.  The oscillator mod
  is folded into inv_std via one DVE multiply with a per-core constant.
Zero bias vectors / unit gamma / zero beta are detected on host at call time
and their device ops are skipped (generic selector-matmul paths are emitted
when the values are non-trivial).
"""
import math
import numpy as np
from contextlib import ExitStack

import concourse.bass as bass
import concourse.tile as tile
from concourse import bacc, mybir
from concourse.bass_utils import run_bass_kernel_spmd

N_CORES = 8
B = 32          # batch
D = 256         # model dim
HIST = 8
HID = 512
N_NEURONS = 64
NPC = N_NEURONS // N_CORES  # 8 neurons per core
IN_DIM = D * (1 + HIST)     # 2304
KC1 = IN_DIM // 128         # 18 contraction chunks for GEMM1
KC2 = HID // 128            # 4 chunks for GEMM2/GEMM3
W3F = D + 1                 # GEMM3 free dim: 256 outputs + row-sum column
GA = 6                      # neurons in the early sqrt group
LN_EPS = 1e-5
FMIN, FMAX = 0.5, 40.0
TICK_INTERVAL = 0.1

f32 = mybir.dt.float32
f16 = mybir.dt.float16

# packed per-neuron row layout for the generic (non-zero bias) path
B1_OFF = 0
B2_OFF = B1_OFF + HID
B3_OFF = B2_OFF + HID
GM_OFF = B3_OFF + W3F
BM_OFF = GM_OFF + D
BVEC_LEN = BM_OFF + D

_CACHE = {}


def _build_program(flags):
    # flags: (b1_zero, b2_zero, b3_zero, gamma_one, beta_zero)
    b1z, b2z, b3z, g1, bz = flags
    nc = bacc.Bacc("TRN2", target_bir_lowering=False, debug=False,
                   num_devices=N_CORES)

    embTd = nc.dram_tensor("embTd", [128, 2, B], f16, kind="ExternalInput").ap()
    wpd = nc.dram_tensor("wpd", [128, 2, D], f16, kind="ExternalInput").ap()
    bpd = nc.dram_tensor("bpd", [128, 2], f32, kind="ExternalInput").ap()
    histd = nc.dram_tensor("histd", [128, HIST * 2], f32, kind="ExternalInput").ap()
    eyed = nc.dram_tensor("eyed", [32, 32], f16, kind="ExternalInput").ap()
    w1d = nc.dram_tensor("w1d", [NPC, 128, KC1, HID], f16, kind="ExternalInput").ap()
    w2d = nc.dram_tensor("w2d", [NPC, 128, KC2, HID], f16, kind="ExternalInput").ap()
    w3d = nc.dram_tensor("w3d", [NPC, 128, KC2, W3F], f16, kind="ExternalInput").ap()
    bvecd = nc.dram_tensor("bvecd", [NPC, BVEC_LEN], f16, kind="ExternalInput").ap()
    sel8d = nc.dram_tensor("sel8d", [NPC, NPC * B], f16, kind="ExternalInput").ap()
    modd = nc.dram_tensor("modd", [B, NPC], f32, kind="ExternalInput").ap()
    out = nc.dram_tensor("out", [B, NPC, D], f32, kind="ExternalOutput").ap()

    GELU = mybir.ActivationFunctionType.Gelu
    SQRT = mybir.ActivationFunctionType.Sqrt
    COPY = mybir.ActivationFunctionType.Copy
    MULT = mybir.AluOpType.mult

    with tile.TileContext(nc) as tc, ExitStack() as ctx:
        cst = ctx.enter_context(tc.tile_pool(name="cst", bufs=1))
        w1p = ctx.enter_context(tc.tile_pool(name="w1p", bufs=8))
        w23p = ctx.enter_context(tc.tile_pool(name="w23p", bufs=8))
        htp = ctx.enter_context(tc.tile_pool(name="htp", bufs=16))
        hp = ctx.enter_context(tc.tile_pool(name="hp", bufs=4))
        ysp = ctx.enter_context(tc.tile_pool(name="ysp", bufs=NPC))
        yop = ctx.enter_context(tc.tile_pool(name="yop", bufs=NPC))
        stp = ctx.enter_context(tc.tile_pool(name="stp", bufs=4))
        scp = ctx.enter_context(tc.tile_pool(name="scp", bufs=2))
        accp = ctx.enter_context(tc.tile_pool(name="accp", bufs=4, space="PSUM"))
        trp = ctx.enter_context(tc.tile_pool(name="trp", bufs=4, space="PSUM"))

        need_sel = not (b1z and b2z and b3z and g1 and bz)

        # ---- inputs. embT/wp lead the fast gpsimd ring (the proj path gates
        # the first GEMM); everything small rides the sync ring.
        embT = cst.tile([128, 2, B], f16, tag="embT")
        nc.gpsimd.dma_start(out=embT[:], in_=embTd)
        wpt = cst.tile([128, 2, D], f16, tag="wpt")
        nc.gpsimd.dma_start(out=wpt[:], in_=wpd)

        eye = cst.tile([32, 32], f16, tag="eye")
        nc.sync.dma_start(out=eye[:], in_=eyed)
        epst = cst.tile([B, 1], f32, tag="epst")
        nc.vector.memset(epst[:], LN_EPS)
        onesb = cst.tile([128, B], f16, tag="onesb")
        nc.vector.memset(onesb[:], 1.0)
        bpt = cst.tile([128, 2], f32, tag="bpt")
        nc.sync.dma_start(out=bpt[:], in_=bpd)
        hist16 = cst.tile([128, HIST * 2], f32, tag="hist16")
        nc.sync.dma_start(out=hist16[:], in_=histd)
        modt = cst.tile([B, NPC], f32, tag="modt")
        nc.sync.dma_start(out=modt[:], in_=modd)
        if need_sel:
            bvec = cst.tile([NPC, BVEC_LEN], f16, tag="bvec")
            nc.sync.dma_start(out=bvec[:], in_=bvecd)
            sel8 = cst.tile([NPC, NPC * B], f16, tag="sel8")
            nc.sync.dma_start(out=sel8[:], in_=sel8d)

            def selcol(n):
                return sel8[:, n * B:(n + 1) * B]

        # ---- weight streaming (gpsimd SWDGE ring) ----
        def dma_w1(n):
            wa = w1p.tile([128, 9, HID], f16, tag="w1")
            nc.gpsimd.dma_start(out=wa[:], in_=w1d[n][:, 0:9, :])
            wb = w1p.tile([128, 9, HID], f16, tag="w1")
            nc.gpsimd.dma_start(out=wb[:], in_=w1d[n][:, 9:18, :])
            return (wa, wb)

        def dma_w2(n):
            w2t = w23p.tile([128, KC2, HID], f16, tag="w2")
            nc.gpsimd.dma_start(out=w2t[:], in_=w2d[n])
            return w2t

        def dma_w3(n):
            w3t = w23p.tile([128, KC2, W3F], f16, tag="w3")
            nc.gpsimd.dma_start(out=w3t[:], in_=w3d[n])
            return w3t

        # ---- x setup: 18 lhsT chunks [128, 32] f16 ----
        xT = []
        for m in range(2):
            pp = trp.tile([128, 32], f32, tag="tr")
            for k in range(2):
                nc.tensor.matmul(pp[:], wpt[:, k, m * 128:(m + 1) * 128],
                                 embT[:, k, :], start=(k == 0), stop=(k == 1))
            xt = cst.tile([128, 32], f16, tag=f"xt{m}")
            nc.vector.tensor_scalar_add(xt[:], pp[:], bpt[:, m:m + 1])
            xT.append(xt)
        histb = cst.tile([128, HIST * 2, B], f16, tag="histb")
        for c in range(HIST * 2):
            nc.vector.tensor_scalar_mul(histb[:, c, :], onesb[:],
                                        hist16[:, c:c + 1])

        def xchunk(k):
            if k < 2:
                return xT[k][:]
            return histb[:, k - 2, :]

        # ---- pipelined per-neuron schedule ----
        ssqA = cst.tile([B, GA], f32, tag="ssqA")
        ssqB = cst.tile([B, NPC - GA], f32, tag="ssqB")
        ycs = {}
        invA = {}
        invB = {}

        def g1_mm(n, w1t):
            p1 = accp.tile([B, HID], f32, tag="acc")
            if not b1z:
                nc.tensor.matmul(p1[:], selcol(n), bvec[:, B1_OFF:B1_OFF + HID],
                                 start=True, stop=False)
            for k in range(KC1):
                nc.tensor.matmul(p1[:], xchunk(k), w1t[k // 9][:, k % 9, :],
                                 start=(b1z and k == 0), stop=(k == KC1 - 1))
            return p1

        def gelu(p):
            h = hp.tile([B, HID], f16, tag="h")
            nc.scalar.activation(h[:], p[:], GELU)
            return h

        def transpose4(h):
            hT = []
            for j in range(KC2):
                pt = trp.tile([128, 32], f16, tag="tr")
                nc.tensor.transpose(pt[:], h[:, j * 128:(j + 1) * 128], eye[:])
                st = htp.tile([128, 32], f16, tag="hT")
                if j % 2 == 0:
                    nc.vector.tensor_copy(st[:], pt[:])
                else:
                    nc.scalar.copy(st[:], pt[:])
                hT.append(st)
            return hT

        def g2_mm(n, w2t, h1T):
            p2 = accp.tile([B, HID], f32, tag="acc")
            if not b2z:
                nc.tensor.matmul(p2[:], selcol(n), bvec[:, B2_OFF:B2_OFF + HID],
                                 start=True, stop=False)
            for j in range(KC2):
                nc.tensor.matmul(p2[:], h1T[j][:], w2t[:, j, :],
                                 start=(b2z and j == 0), stop=(j == KC2 - 1))
            return p2

        def g3_mm(n, w3t, h2T):
            p3 = accp.tile([B, W3F], f32, tag="acc")
            if not b3z:
                nc.tensor.matmul(p3[:], selcol(n), bvec[:, B3_OFF:B3_OFF + W3F],
                                 start=True, stop=False)
            for j in range(KC2):
                nc.tensor.matmul(p3[:], h2T[j][:], w3t[:, j, :],
                                 start=(b3z and j == 0), stop=(j == KC2 - 1))
            return p3

        def ln_stats(n, p3):
            nmu = stp.tile([B, 1], f32, tag="st")
            nc.vector.tensor_scalar_mul(nmu[:], p3[:, D:D + 1], -1.0 / D)
            yc = ysp.tile([B, D], f32, tag="ys")
            nc.vector.tensor_scalar_add(yc[:], p3[:, 0:D], nmu[:])
            sq = scp.tile([B, D], f32, tag="sq")
            nc.vector.tensor_tensor(sq[:], yc[:], yc[:], MULT)
            if n < GA:
                nc.vector.tensor_reduce(ssqA[:, n:n + 1], sq[:],
                                        mybir.AxisListType.X, mybir.AluOpType.add)
            else:
                nc.vector.tensor_reduce(ssqB[:, n - GA:n - GA + 1], sq[:],
                                        mybir.AxisListType.X, mybir.AluOpType.add)
            ycs[n] = yc

        def sqrt_group(ssq, width, inv_map, base):
            std = stp.tile([B, width], f32, tag=f"std{base}")
            nc.scalar.activation(std[:], ssq[:], SQRT, bias=epst[:], scale=1.0 / D)
            inv = stp.tile([B, width], f32, tag=f"inv{base}")
            nc.vector.reciprocal(inv[:], std[:])
            if g1 and bz:
                nc.vector.tensor_tensor(inv[:], inv[:],
                                        modt[:, base:base + width], MULT)
            for i in range(width):
                inv_map[base + i] = inv[:, i:i + 1]

        def tail(n, inv_n, dve, dma_eng):
            yc = ycs[n]
            if g1 and bz:
                yo = yop.tile([B, D], f32, tag="yo")
                if dve:
                    nc.vector.tensor_scalar_mul(yo[:], yc[:], inv_n)
                else:
                    nc.scalar.activation(yo[:], yc[:], COPY, scale=inv_n)
            else:
                gb = trp.tile([B, 2 * D], f32, tag="tr")
                nc.tensor.matmul(gb[:, 0:D], selcol(n),
                                 bvec[:, GM_OFF:GM_OFF + D], start=True, stop=True)
                nc.tensor.matmul(gb[:, D:2 * D], selcol(n),
                                 bvec[:, BM_OFF:BM_OFF + D], start=True, stop=True)
                yg = yop.tile([B, D], f32, tag="yo")
                nc.vector.scalar_tensor_tensor(yg[:], yc[:], inv_n, gb[:, 0:D],
                                               MULT, MULT)
                yo = yop.tile([B, D], f32, tag="yo")
                nc.vector.tensor_add(yo[:], yg[:], gb[:, D:2 * D])
            dma_eng.dma_start(out=out[:, n, :], in_=yo[:])

        # pipeline: step n issues G1(n) and retires neuron n-1
        h1Ts = {}
        h2Ts = {}
        w2ts = {}
        w3ts = {}
        for n in range(NPC):
            w1t = dma_w1(n)
            w2ts[n] = dma_w2(n)
            w3ts[n] = dma_w3(n)
            p1 = g1_mm(n, w1t)
            h1 = gelu(p1)
            if n >= 1:
                p2 = g2_mm(n - 1, w2ts[n - 1], h1Ts[n - 1])
                h2 = gelu(p2)
            h1Ts[n] = transpose4(h1)
            if n >= 1:
                h2Ts[n - 1] = transpose4(h2)
                p3 = g3_mm(n - 1, w3ts[n - 1], h2Ts[n - 1])
                ln_stats(n - 1, p3)
            if n - 1 == GA - 1:
                sqrt_group(ssqA, GA, invA, 0)
                for i in range(GA):
                    tail(i, invA[i], dve=(i % 2 == 0), dma_eng=nc.sync)
        # epilogue: retire neuron 7
        L = NPC - 1
        p2 = g2_mm(L, w2ts[L], h1Ts[L])
        h2 = gelu(p2)
        h2Ts[L] = transpose4(h2)
        p3 = g3_mm(L, w3ts[L], h2Ts[L])
        ln_stats(L, p3)
        sqrt_group(ssqB, NPC - GA, invB, GA)
        tail(GA, invB[GA], dve=False, dma_eng=nc.sync)
        tail(L, invB[L], dve=True, dma_eng=nc.gpsimd)

    nc.compile()
    return nc


def _get_program(flags):
    if flags not in _CACHE:
        _CACHE[flags] = _build_program(flags)
    return _CACHE[flags]


def _prep(input_embedding, pre_activations, Wp, bp, W1, b1, W2, b2, W3, b3,
          gamma, beta, tick):
    emb = np.asarray(input_embedding, dtype=np.float32)
    hist = np.asarray(pre_activations, dtype=np.float32)
    Wp = np.asarray(Wp, dtype=np.float32)
    bp = np.asarray(bp, dtype=np.float32)
    W1 = np.asarray(W1, dtype=np.float32)
    b1 = np.asarray(b1, dtype=np.float32)
    W2 = np.asarray(W2, dtype=np.float32)
    b2 = np.asarray(b2, dtype=np.float32)
    W3 = np.asarray(W3, dtype=np.float32)
    b3 = np.asarray(b3, dtype=np.float32)
    gamma = np.asarray(gamma, dtype=np.float32)
    beta = np.asarray(beta, dtype=np.float32)

    # oscillator modulation: deterministic in (tick, n); folded into inv_std
    # (gamma==1, beta==0) or into gamma*mod / beta*mod rows otherwise
    i = np.arange(N_NEURONS, dtype=np.float64)
    freq = FMIN * (FMAX / FMIN) ** (i / (N_NEURONS - 1))
    phase = np.mod(i * 2.3571, 2.0 * math.pi)
    t = float(np.asarray(tick)) * TICK_INTERVAL
    mod = (1.0 + 0.5 * np.sin(2.0 * math.pi * freq * t + phase)).astype(np.float32)

    b1z = not np.any(b1)
    b2z = not np.any(b2)
    b3z = not np.any(b3)
    g1 = bool(np.all(gamma == 1.0))
    bz = not np.any(beta)

    # fp16 weight layouts: (n, partition, k_chunk, free) with contiguous
    # per-partition runs; W3 gains a row-sum column so the GEMM also
    # produces sum_d(y) for the LayerNorm mean
    W1r = np.ascontiguousarray(
        W1.reshape(N_NEURONS, KC1, 128, HID).transpose(0, 2, 1, 3)).astype(np.float16)
    W2r = np.ascontiguousarray(
        W2.reshape(N_NEURONS, KC2, 128, HID).transpose(0, 2, 1, 3)).astype(np.float16)
    W3a = np.concatenate([W3, W3.sum(axis=2, keepdims=True)], axis=2)
    W3r = np.ascontiguousarray(
        W3a.reshape(N_NEURONS, KC2, 128, W3F).transpose(0, 2, 1, 3)).astype(np.float16)

    embT = np.ascontiguousarray(emb.T.reshape(2, 128, B).transpose(1, 0, 2)).astype(np.float16)
    wpt = np.ascontiguousarray(Wp.reshape(2, 128, D).transpose(1, 0, 2)).astype(np.float16)
    bpd = np.ascontiguousarray(bp.reshape(2, 128).T)
    hist16 = np.ascontiguousarray(hist.reshape(-1).reshape(16, 128).T)  # [128, 16]
    eyed = np.eye(32, dtype=np.float16)

    gm = (gamma * mod[:, None]).astype(np.float32)
    bm = (beta * mod[:, None]).astype(np.float32)
    b3a = np.concatenate([b3, b3.sum(axis=1, keepdims=True)], axis=1)
    sel8 = np.zeros((NPC, NPC * B), dtype=np.float16)
    for n in range(NPC):
        sel8[n, n * B:(n + 1) * B] = 1.0

    in_maps = []
    for c in range(N_CORES):
        s = slice(c * NPC, (c + 1) * NPC)
        bvec = np.concatenate([b1[s], b2[s], b3a[s], gm[s], bm[s]],
                              axis=1).astype(np.float16)
        modrow = np.broadcast_to(mod[c * NPC:(c + 1) * NPC][None, :],
                                 (B, NPC)).astype(np.float32)
        in_maps.append({
            "modd": np.ascontiguousarray(modrow),
            "embTd": embT,
            "wpd": wpt,
            "bpd": bpd,
            "histd": hist16,
            "eyed": eyed,
            "w1d": W1r[s],
            "w2d": W2r[s],
            "w3d": W3r[s],
            "bvecd": np.ascontiguousarray(bvec),
            "sel8d": sel8,
        })
    flags = (b1z, b2z, b3z, g1, bz)
    return in_maps, flags


def run(inputs, trace=False):
    in_maps, flags = _prep(**inputs)
    nc = _get_program(flags)
    br = run_bass_kernel_spmd(nc, in_maps, core_ids=list(range(N_CORES)),
                              trace=trace)
    out = np.concatenate([r["out"] for r in br.results], axis=1)
    return np.ascontiguousarray(out, dtype=np.float32), br


def kernel(**inputs) -> np.ndarray:
    out, _ = run(inputs, trace=False)
    return out


# revision 8
# speedup vs baseline: 1.6229x; 1.0631x over previous
"""NeuronPool (moe_routing) Trainium2 kernel.

Expert-parallel over 8 NeuronCores: core c computes neurons [8c, 8c+8) for the
full batch, host concatenates along the neuron axis.

The kernel is HBM-bound: ~25 MB of fp16 weights stream per core at ~420 GB/s
(weights cast on host; matmuls run at the full 1 cycle/row PE rate, ~5e-4
relative rounding).  The per-neuron MLPs are software-pipelined one neuron
deep so the PE never waits on gelu: step n runs GEMM1(n) while GEMM2/GEMM3 of
neuron n-1 fill the gelu/transpose latency.

  x = [proj | hist]: proj = Wp.T @ embT on the PE; hist chunks broadcast
      across batch on the DVE.  W1/W2 weights stream on the fast gpsimd SWDGE
      ring (16 DMA engines); small constants ride the sync ring.
  step n:  G1(n) 18mm -> [gelu h1(n)] ; G2(n-1) ; tr4 h1(n) ; [gelu h2(n-1)]
           tr4 h2(n-1) ; G3(n-1) -> LN stats(n-1) on DVE
  GEMM3 output is [32,257]: col 256 = row-sums of W3 (host-augmented), so the
  LayerNorm mean is a free by-product of the GEMM.
  LN scale: sqrt over neurons {0..5} runs while neurons 6-7 compute (their
  outputs drain early); only {6,7} remain in the tail.  The oscillator mod
  is folded into inv_std via one DVE multiply with a per-core constant.
Zero bias vectors / unit gamma / zero beta are detected on host at call time
and their device ops are skipped (generic selector-matmul paths are emitted
when the values are non-trivial).
"""
import math
import numpy as np
from contextlib import ExitStack

import concourse.bass as bass
import concourse.tile as tile
from concourse import bacc, mybir
from concourse.bass_utils import run_bass_kernel_spmd

N_CORES = 8
B = 32          # batch
D = 256         # model dim
HIST = 8
HID = 512
N_NEURONS = 64
NPC = N_NEURONS // N_CORES  # 8 neurons per core
IN_DIM = D * (1 + HIST)     # 2304
KC1 = IN_DIM // 128         # 18 contraction chunks for GEMM1
KC2 = HID // 128            # 4 chunks for GEMM2/GEMM3
W3F = D + 1                 # GEMM3 free dim: 256 outputs + row-sum column
GA = 6                      # neurons in the early sqrt group
LN_EPS = 1e-5
FMIN, FMAX = 0.5, 40.0
TICK_INTERVAL = 0.1

f32 = mybir.dt.float32
f16 = mybir.dt.float16

# packed per-neuron row layout for the generic (non-zero bias) path
B1_OFF = 0
B2_OFF = B1_OFF + HID
B3_OFF = B2_OFF + HID
GM_OFF = B3_OFF + W3F
BM_OFF = GM_OFF + D
BVEC_LEN = BM_OFF + D

_CACHE = {}


def _build_program(flags):
    # flags: (b1_zero, b2_zero, b3_zero, gamma_one, beta_zero)
    b1z, b2z, b3z, g1, bz = flags
    nc = bacc.Bacc("TRN2", target_bir_lowering=False, debug=False,
                   num_devices=N_CORES)

    embTd = nc.dram_tensor("embTd", [128, 2, B], f16, kind="ExternalInput").ap()
    wpd = nc.dram_tensor("wpd", [128, 2, D], f16, kind="ExternalInput").ap()
    bpd = nc.dram_tensor("bpd", [128, 2], f32, kind="ExternalInput").ap()
    histd = nc.dram_tensor("histd", [128, HIST * 2], f32, kind="ExternalInput").ap()
    eyed = nc.dram_tensor("eyed", [32, 32], f16, kind="ExternalInput").ap()
    w1d = nc.dram_tensor("w1d", [NPC, 128, KC1, HID], f16, kind="ExternalInput").ap()
    w2d = nc.dram_tensor("w2d", [NPC, 128, KC2, HID], f16, kind="ExternalInput").ap()
    w3d = nc.dram_tensor("w3d", [NPC, 128, KC2, W3F], f16, kind="ExternalInput").ap()
    bvecd = nc.dram_tensor("bvecd", [NPC, BVEC_LEN], f16, kind="ExternalInput").ap()
    sel8d = nc.dram_tensor("sel8d", [NPC, NPC * B], f16, kind="ExternalInput").ap()
    modd = nc.dram_tensor("modd", [B, NPC], f32, kind="ExternalInput").ap()
    out = nc.dram_tensor("out", [B, NPC, D], f32, kind="ExternalOutput").ap()

    GELU = mybir.ActivationFunctionType.Gelu
    SQRT = mybir.ActivationFunctionType.Sqrt
    COPY = mybir.ActivationFunctionType.Copy
    MULT = mybir.AluOpType.mult

    with tile.TileContext(nc) as tc, ExitStack() as ctx:
        cst = ctx.enter_context(tc.tile_pool(name="cst", bufs=1))
        w1p = ctx.enter_context(tc.tile_pool(name="w1p", bufs=8))
        w23p = ctx.enter_context(tc.tile_pool(name="w23p", bufs=8))
        htp = ctx.enter_context(tc.tile_pool(name="htp", bufs=16))
        hp = ctx.enter_context(tc.tile_pool(name="hp", bufs=4))
        ysp = ctx.enter_context(tc.tile_pool(name="ysp", bufs=NPC))
        yop = ctx.enter_context(tc.tile_pool(name="yop", bufs=NPC))
        stp = ctx.enter_context(tc.tile_pool(name="stp", bufs=4))
        scp = ctx.enter_context(tc.tile_pool(name="scp", bufs=2))
        accp = ctx.enter_context(tc.tile_pool(name="accp", bufs=4, space="PSUM"))
        trp = ctx.enter_context(tc.tile_pool(name="trp", bufs=4, space="PSUM"))

        need_sel = not (b1z and b2z and b3z and g1 and bz)

        # ---- inputs. embT/wp lead the fast gpsimd ring (the proj path gates
        # the first GEMM); everything small rides the sync ring.
        embT = cst.tile([128, 2, B], f16, tag="embT")
        nc.gpsimd.dma_start(out=embT[:], in_=embTd)
        wpt = cst.tile([128, 2, D], f16, tag="wpt")
        nc.gpsimd.dma_start(out=wpt[:], in_=wpd)

        eye = cst.tile([32, 32], f16, tag="eye")
        nc.sync.dma_start(out=eye[:], in_=eyed)
        epst = cst.tile([B, 1], f32, tag="epst")
        nc.vector.memset(epst[:], LN_EPS)
        onesb = cst.tile([128, B], f16, tag="onesb")
        nc.vector.memset(onesb[:], 1.0)
        bpt = cst.tile([128, 2], f32, tag="bpt")
        nc.sync.dma_start(out=bpt[:], in_=bpd)
        hist16 = cst.tile([128, HIST * 2], f32, tag="hist16")
        nc.sync.dma_start(out=hist16[:], in_=histd)
        modt = cst.tile([B, NPC], f32, tag="modt")
        nc.sync.dma_start(out=modt[:], in_=modd)
        if need_sel:
            bvec = cst.tile([NPC, BVEC_LEN], f16, tag="bvec")
            nc.sync.dma_start(out=bvec[:], in_=bvecd)
            sel8 = cst.tile([NPC, NPC * B], f16, tag="sel8")
            nc.sync.dma_start(out=sel8[:], in_=sel8d)

            def selcol(n):
                return sel8[:, n * B:(n + 1) * B]

        # ---- weight streaming (gpsimd SWDGE ring) ----
        def dma_w1(n):
            wa = w1p.tile([128, 9, HID], f16, tag="w1")
            nc.gpsimd.dma_start(out=wa[:], in_=w1d[n][:, 0:9, :])
            wb = w1p.tile([128, 9, HID], f16, tag="w1")
            nc.gpsimd.dma_start(out=wb[:], in_=w1d[n][:, 9:18, :])
            return (wa, wb)

        def dma_w2(n):
            w2t = w23p.tile([128, KC2, HID], f16, tag="w2")
            nc.gpsimd.dma_start(out=w2t[:], in_=w2d[n])
            return w2t

        def dma_w3(n):
            w3t = w23p.tile([128, KC2, W3F], f16, tag="w3")
            nc.gpsimd.dma_start(out=w3t[:], in_=w3d[n])
            return w3t

        # ---- x setup: 18 lhsT chunks [128, 32] f16 ----
        xT = []
        for m in range(2):
            pp = trp.tile([128, 32], f32, tag="tr")
            for k in range(2):
                nc.tensor.matmul(pp[:], wpt[:, k, m * 128:(m + 1) * 128],
                                 embT[:, k, :], start=(k == 0), stop=(k == 1))
            xt = cst.tile([128, 32], f16, tag=f"xt{m}")
            nc.vector.tensor_scalar_add(xt[:], pp[:], bpt[:, m:m + 1])
            xT.append(xt)
        histb = cst.tile([128, HIST * 2, B], f16, tag="histb")
        for c in range(HIST * 2):
            nc.vector.tensor_scalar_mul(histb[:, c, :], onesb[:],
                                        hist16[:, c:c + 1])

        def xchunk(k):
            if k < 2:
                return xT[k][:]
            return histb[:, k - 2, :]

        # ---- pipelined per-neuron schedule ----
        ssqA = cst.tile([B, GA], f32, tag="ssqA")
        ssqB = cst.tile([B, NPC - GA], f32, tag="ssqB")
        ycs = {}
        invA = {}
        invB = {}

        def g1_mm(n, w1t):
            p1 = accp.tile([B, HID], f32, tag="acc")
            if not b1z:
                nc.tensor.matmul(p1[:], selcol(n), bvec[:, B1_OFF:B1_OFF + HID],
                                 start=True, stop=False)
            for k in range(KC1):
                nc.tensor.matmul(p1[:], xchunk(k), w1t[k // 9][:, k % 9, :],
                                 start=(b1z and k == 0), stop=(k == KC1 - 1))
            return p1

        def gelu(p):
            h = hp.tile([B, HID], f16, tag="h")
            nc.scalar.activation(h[:], p[:], GELU)
            return h

        def transpose4(h):
            hT = []
            for j in range(KC2):
                pt = trp.tile([128, 32], f16, tag="tr")
                nc.tensor.transpose(pt[:], h[:, j * 128:(j + 1) * 128], eye[:])
                st = htp.tile([128, 32], f16, tag="hT")
                if j % 2 == 0:
                    nc.vector.tensor_copy(st[:], pt[:])
                else:
                    nc.scalar.copy(st[:], pt[:])
                hT.append(st)
            return hT

        def g2_mm(n, w2t, h1T):
            p2 = accp.tile([B, HID], f32, tag="acc")
            if not b2z:
                nc.tensor.matmul(p2[:], selcol(n), bvec[:, B2_OFF:B2_OFF + HID],
                                 start=True, stop=False)
            for j in range(KC2):
                nc.tensor.matmul(p2[:], h1T[j][:], w2t[:, j, :],
                                 start=(b2z and j == 0), stop=(j == KC2 - 1))
            return p2

        def g3_mm(n, w3t, h2T):
            p3 = accp.tile([B, W3F], f32, tag="acc")
            if not b3z:
                nc.tensor.matmul(p3[:], selcol(n), bvec[:, B3_OFF:B3_OFF + W3F],
                                 start=True, stop=False)
            for j in range(KC2):
                nc.tensor.matmul(p3[:], h2T[j][:], w3t[:, j, :],
                                 start=(b3z and j == 0), stop=(j == KC2 - 1))
            return p3

        def ln_stats(n, p3):
            nmu = stp.tile([B, 1], f32, tag="st")
            nc.vector.tensor_scalar_mul(nmu[:], p3[:, D:D + 1], -1.0 / D)
            yc = ysp.tile([B, D], f32, tag="ys")
            nc.vector.tensor_scalar_add(yc[:], p3[:, 0:D], nmu[:])
            sq = scp.tile([B, D], f32, tag="sq")
            nc.vector.tensor_tensor(sq[:], yc[:], yc[:], MULT)
            if n < GA:
                nc.vector.tensor_reduce(ssqA[:, n:n + 1], sq[:],
                                        mybir.AxisListType.X, mybir.AluOpType.add)
            else:
                nc.vector.tensor_reduce(ssqB[:, n - GA:n - GA + 1], sq[:],
                                        mybir.AxisListType.X, mybir.AluOpType.add)
            ycs[n] = yc

        def sqrt_group(ssq, width, inv_map, base):
            std = stp.tile([B, width], f32, tag=f"std{base}")
            nc.scalar.activation(std[:], ssq[:], SQRT, bias=epst[:], scale=1.0 / D)
            inv = stp.tile([B, width], f32, tag=f"inv{base}")
            nc.vector.reciprocal(inv[:], std[:])
            if g1 and bz:
                nc.vector.tensor_tensor(inv[:], inv[:],
                                        modt[:, base:base + width], MULT)
            for i in range(width):
                inv_map[base + i] = inv[:, i:i + 1]

        def tail(n, inv_n, dve, dma_eng):
            yc = ycs[n]
            if g1 and bz:
                yo = yop.tile([B, D], f32, tag="yo")
                if dve:
                    nc.vector.tensor_scalar_mul(yo[:], yc[:], inv_n)
                else:
                    nc.scalar.activation(yo[:], yc[:], COPY, scale=inv_n)
            else:
                gb = trp.tile([B, 2 * D], f32, tag="tr")
                nc.tensor.matmul(gb[:, 0:D], selcol(n),
                                 bvec[:, GM_OFF:GM_OFF + D], start=True, stop=True)
                nc.tensor.matmul(gb[:, D:2 * D], selcol(n),
                                 bvec[:, BM_OFF:BM_OFF + D], start=True, stop=True)
                yg = yop.tile([B, D], f32, tag="yo")
                nc.vector.scalar_tensor_tensor(yg[:], yc[:], inv_n, gb[:, 0:D],
                                               MULT, MULT)
                yo = yop.tile([B, D], f32, tag="yo")
                nc.vector.tensor_add(yo[:], yg[:], gb[:, D:2 * D])
            dma_eng.dma_start(out=out[:, n, :], in_=yo[:])

        # pipeline: step n retires neuron n-1 through GEMM2/3 while GEMM1(n)
        # runs; emission order matches the intended per-engine execution
        # order (G2(n-1), G1(n), tr h2(n-1), G3(n-1), tr h1(n)) so the gelus
        # retire in the order the PE consumes them
        h1Ts = {}
        h2Ts = {}
        w2ts = {}
        w3ts = {}
        for n in range(NPC):
            w1t = dma_w1(n)
            w2ts[n] = dma_w2(n)
            w3ts[n] = dma_w3(n)
            if n >= 1:
                p2 = g2_mm(n - 1, w2ts[n - 1], h1Ts[n - 1])
                h2 = gelu(p2)
            p1 = g1_mm(n, w1t)
            h1 = gelu(p1)
            if n >= 1:
                h2Ts[n - 1] = transpose4(h2)
                p3 = g3_mm(n - 1, w3ts[n - 1], h2Ts[n - 1])
                ln_stats(n - 1, p3)
            h1Ts[n] = transpose4(h1)
            if n - 1 == GA - 1:
                sqrt_group(ssqA, GA, invA, 0)
                for i in range(GA):
                    tail(i, invA[i], dve=(i % 2 == 0), dma_eng=nc.sync)
        # epilogue: retire neuron 7
        L = NPC - 1
        p2 = g2_mm(L, w2ts[L], h1Ts[L])
        h2 = gelu(p2)
        h2Ts[L] = transpose4(h2)
        p3 = g3_mm(L, w3ts[L], h2Ts[L])
        ln_stats(L, p3)
        sqrt_group(ssqB, NPC - GA, invB, GA)
        tail(GA, invB[GA], dve=False, dma_eng=nc.sync)
        tail(L, invB[L], dve=True, dma_eng=nc.gpsimd)

    nc.compile()
    return nc


def _get_program(flags):
    if flags not in _CACHE:
        _CACHE[flags] = _build_program(flags)
    return _CACHE[flags]


def _prep(input_embedding, pre_activations, Wp, bp, W1, b1, W2, b2, W3, b3,
          gamma, beta, tick):
    emb = np.asarray(input_embedding, dtype=np.float32)
    hist = np.asarray(pre_activations, dtype=np.float32)
    Wp = np.asarray(Wp, dtype=np.float32)
    bp = np.asarray(bp, dtype=np.float32)
    W1 = np.asarray(W1, dtype=np.float32)
    b1 = np.asarray(b1, dtype=np.float32)
    W2 = np.asarray(W2, dtype=np.float32)
    b2 = np.asarray(b2, dtype=np.float32)
    W3 = np.asarray(W3, dtype=np.float32)
    b3 = np.asarray(b3, dtype=np.float32)
    gamma = np.asarray(gamma, dtype=np.float32)
    beta = np.asarray(beta, dtype=np.float32)

    # oscillator modulation: deterministic in (tick, n); folded into inv_std
    # (gamma==1, beta==0) or into gamma*mod / beta*mod rows otherwise
    i = np.arange(N_NEURONS, dtype=np.float64)
    freq = FMIN * (FMAX / FMIN) ** (i / (N_NEURONS - 1))
    phase = np.mod(i * 2.3571, 2.0 * math.pi)
    t = float(np.asarray(tick)) * TICK_INTERVAL
    mod = (1.0 + 0.5 * np.sin(2.0 * math.pi * freq * t + phase)).astype(np.float32)

    b1z = not np.any(b1)
    b2z = not np.any(b2)
    b3z = not np.any(b3)
    g1 = bool(np.all(gamma == 1.0))
    bz = not np.any(beta)

    # fp16 weight layouts: (n, partition, k_chunk, free) with contiguous
    # per-partition runs; W3 gains a row-sum column so the GEMM also
    # produces sum_d(y) for the LayerNorm mean
    W1r = np.ascontiguousarray(
        W1.reshape(N_NEURONS, KC1, 128, HID).transpose(0, 2, 1, 3)).astype(np.float16)
    W2r = np.ascontiguousarray(
        W2.reshape(N_NEURONS, KC2, 128, HID).transpose(0, 2, 1, 3)).astype(np.float16)
    W3a = np.concatenate([W3, W3.sum(axis=2, keepdims=True)], axis=2)
    W3r = np.ascontiguousarray(
        W3a.reshape(N_NEURONS, KC2, 128, W3F).transpose(0, 2, 1, 3)).astype(np.float16)

    embT = np.ascontiguousarray(emb.T.reshape(2, 128, B).transpose(1, 0, 2)).astype(np.float16)
    wpt = np.ascontiguousarray(Wp.reshape(2, 128, D).transpose(1, 0, 2)).astype(np.float16)
    bpd = np.ascontiguousarray(bp.reshape(2, 128).T)
    hist16 = np.ascontiguousarray(hist.reshape(-1).reshape(16, 128).T)  # [128, 16]
    eyed = np.eye(32, dtype=np.float16)

    gm = (gamma * mod[:, None]).astype(np.float32)
    bm = (beta * mod[:, None]).astype(np.float32)
    b3a = np.concatenate([b3, b3.sum(axis=1, keepdims=True)], axis=1)
    sel8 = np.zeros((NPC, NPC * B), dtype=np.float16)
    for n in range(NPC):
        sel8[n, n * B:(n + 1) * B] = 1.0

    in_maps = []
    for c in range(N_CORES):
        s = slice(c * NPC, (c + 1) * NPC)
        bvec = np.concatenate([b1[s], b2[s], b3a[s], gm[s], bm[s]],
                              axis=1).astype(np.float16)
        modrow = np.broadcast_to(mod[c * NPC:(c + 1) * NPC][None, :],
                                 (B, NPC)).astype(np.float32)
        in_maps.append({
            "modd": np.ascontiguousarray(modrow),
            "embTd": embT,
            "wpd": wpt,
            "bpd": bpd,
            "histd": hist16,
            "eyed": eyed,
            "w1d": W1r[s],
            "w2d": W2r[s],
            "w3d": W3r[s],
            "bvecd": np.ascontiguousarray(bvec),
            "sel8d": sel8,
        })
    flags = (b1z, b2z, b3z, g1, bz)
    return in_maps, flags


def run(inputs, trace=False):
    in_maps, flags = _prep(**inputs)
    nc = _get_program(flags)
    br = run_bass_kernel_spmd(nc, in_maps, core_ids=list(range(N_CORES)),
                              trace=trace)
    out = np.concatenate([r["out"] for r in br.results], axis=1)
    return np.ascontiguousarray(out, dtype=np.float32), br


def kernel(**inputs) -> np.ndarray:
    out, _ = run(inputs, trace=False)
    return out
